# revision 1
# baseline (speedup 1.0000x reference)
"""GQA attention kernel for Trainium2, 8-core SPMD.

Sharding: tensor-parallel=4 over kv-head pairs x data-parallel=2 over batch.
Each core: one batch, 8 q-heads, 2 kv-heads, full 2048-token sequence.
Host pre-transposes activations to [hidden, seq] so every matmul is native:
  - Q/K projections produce [d, s] (rope applied in-place via a PE
    half-swap permutation matmul + DVE combine with sign-folded sin table)
  - scores^T [k, q] = K_tile^T @ Q  (softmax reduction over partitions via
    ones-matmul on PE; exp on ACT directly out of PSUM with fused 1/sqrt(d)
    scale; no max-subtraction needed since |score| <~ 10)
  - attn^T [d, q] = V_tile^T @ exp  accumulated over k-chunks in PSUM
  - O partial = attn^T stacked as [f, q] feeding row-sharded Wo
Host sums the 4 TP partials per batch.
All matmuls bf16 inputs / fp32 PSUM accumulation.
"""
import numpy as np
import ml_dtypes

import concourse.bacc as bacc
import concourse.bass as bass
import concourse.tile as tile
from concourse import mybir
from concourse.bass_utils import run_bass_kernel_spmd

BF = mybir.dt.bfloat16
F32 = mybir.dt.float32
BF_NP = np.dtype(ml_dtypes.bfloat16)

# full-problem constants
B, S, HIDDEN = 2, 2048, 4096
NUM_HEADS, NUM_KV_HEADS, HEAD_DIM = 32, 8, 128
GROUPS = NUM_HEADS // NUM_KV_HEADS
ROPE_THETA = 10000.0
TP = 4  # shards over kv-head pairs

FULL_CFG = dict(S=2048, HID=4096, NQ=8, NKV=2, SB=512, QC=512)


def build_nc(cfg):
    S_, HID, NQ, NKV, SB, QC = (cfg[k] for k in ("S", "HID", "NQ", "NKV", "SB", "QC"))
    D = 128
    HC = HID // 128          # hidden chunks (contraction tiles)
    NB = S_ // SB            # phase-1 token blocks
    NQC = S_ // QC           # attention q chunks
    KT = S_ // 128           # k-token tiles
    DV = NKV * 128           # local v width
    NO = HID // 512          # O-proj output chunks
    scale = 1.0 / np.sqrt(128.0)

    nc = bacc.Bacc("TRN2", target_bir_lowering=False, debug=False)
    xt = nc.dram_tensor("xt", (HID, S_), BF, kind="ExternalInput").ap()
    wq = nc.dram_tensor("wq", (HC, NQ, 128, 128), BF, kind="ExternalInput").ap()
    wk = nc.dram_tensor("wk", (HC, NKV, 128, 128), BF, kind="ExternalInput").ap()
    wv = nc.dram_tensor("wv", (HC, 128, DV), BF, kind="ExternalInput").ap()
    wo = nc.dram_tensor("wo", (NQ, NO, 128, 512), BF, kind="ExternalInput").ap()
    cosd = nc.dram_tensor("cos", (128, S_), BF, kind="ExternalInput").ap()
    sind = nc.dram_tensor("sin", (128, S_), BF, kind="ExternalInput").ap()
    rmatd = nc.dram_tensor("rmat", (128, 128), BF, kind="ExternalInput").ap()
    o = nc.dram_tensor("o", (S_, HID), F32, kind="ExternalOutput").ap()

    with tile.TileContext(nc) as tc:
        with tc.tile_pool(name="cons", bufs=1) as cons, \
             tc.tile_pool(name="big", bufs=1) as big:
            cos_sb = cons.tile([128, S_], BF, name="cos_sb")
            sin_sb = cons.tile([128, S_], BF, name="sin_sb")
            r_sb = cons.tile([128, 128], BF, name="r_sb")
            ones_sb = cons.tile([128, 1], BF, name="ones_sb")
            nc.sync.dma_start(out=cos_sb, in_=cosd)
            nc.sync.dma_start(out=sin_sb, in_=sind)
            nc.sync.dma_start(out=r_sb, in_=rmatd)
            nc.vector.memset(ones_sb, 1.0)

            q_sb = big.tile([128, NQ, S_], BF, name="q_sb")
            k_sb = big.tile([128, NKV, S_], BF, name="k_sb")
            v_sb = big.tile([128, KT, DV], BF, name="v_sb")
            wv_sb = big.tile([128, HC, DV], BF, name="wv_sb")
            nc.sync.dma_start(out=wv_sb, in_=wv.rearrange("c p v -> p c v"))

            xt_r = xt.rearrange("(c p) s -> p c s", p=128)

            # ---------------- phase 1: projections + rope ----------------
            with tc.tile_pool(name="xp", bufs=2) as xp, \
                 tc.tile_pool(name="wp", bufs=3) as wp, \
                 tc.tile_pool(name="rt", bufs=4) as rt, \
                 tc.tile_pool(name="pp", bufs=2, space="PSUM") as pp, \
                 tc.tile_pool(name="rp", bufs=2, space="PSUM") as rp:
                for sb_i in range(NB):
                    ssl = slice(sb_i * SB, (sb_i + 1) * SB)
                    xt_t = xp.tile([128, HC, SB], BF, name="xt_t")
                    nc.sync.dma_start(out=xt_t, in_=xt_r[:, :, ssl])

                    # Q then K projections, each with rope
                    for which, nheads, wten, dst in (
                        ("q", NQ, wq, q_sb), ("k", NKV, wk, k_sb)):
                        for h in range(nheads):
                            ps = pp.tile([128, SB], F32, name="ps_proj")
                            wslab = wp.tile([128, HC, 128], BF, name="w_slab")
                            nc.sync.dma_start(
                                out=wslab,
                                in_=wten[:, h].rearrange("c p m -> p c m"))
                            for c in range(HC):
                                nc.tensor.matmul(ps, wslab[:, c, :], xt_t[:, c, :],
                                                 start=(c == 0), stop=(c == HC - 1))
                            # rope: out = ps*cos + (R@ps)*sin_signed
                            qbf = rt.tile([128, SB], BF, name="rope_bf")
                            nc.scalar.activation(out=qbf, in_=ps,
                                                 func=mybir.ActivationFunctionType.Copy)
                            rot = rp.tile([128, SB], F32, name="rot_ps")
                            nc.tensor.matmul(rot, r_sb, qbf, start=True, stop=True)
                            t1 = rt.tile([128, SB], F32, name="rope_t1")
                            t2 = rt.tile([128, SB], F32, name="rope_t2")
                            nc.vector.tensor_mul(t1, ps, cos_sb[:, ssl])
                            nc.vector.tensor_mul(t2, rot, sin_sb[:, ssl])
                            nc.vector.tensor_add(dst[:, h, ssl], t1, t2)

                    # V projection (natural [tok, d] layout)
                    for tt in range(SB // 128):
                        ps = pp.tile([128, DV], F32, name="ps_v")
                        for c in range(HC):
                            nc.tensor.matmul(ps, xt_t[:, c, tt * 128:(tt + 1) * 128],
                                             wv_sb[:, c, :],
                                             start=(c == 0), stop=(c == HC - 1))
                        nc.scalar.activation(out=v_sb[:, sb_i * (SB // 128) + tt, :],
                                             in_=ps,
                                             func=mybir.ActivationFunctionType.Copy)

            # ------------- phase 2+3: attention + output projection -------------
            with tc.tile_pool(name="aq", bufs=2) as aq, \
                 tc.tile_pool(name="ep", bufs=4) as ep, \
                 tc.tile_pool(name="rb", bufs=2) as rb, \
                 tc.tile_pool(name="ob", bufs=2) as ob, \
                 tc.tile_pool(name="wob", bufs=3) as wob, \
                 tc.tile_pool(name="sp", bufs=3, space="PSUM") as sp, \
                 tc.tile_pool(name="ap_", bufs=2, space="PSUM") as ap_, \
                 tc.tile_pool(name="dp", bufs=2, space="PSUM") as dp, \
                 tc.tile_pool(name="op", bufs=1, space="PSUM") as op:
                for qc in range(NQC):
                    qsl = slice(qc * QC, (qc + 1) * QC)
                    at_qc = aq.tile([128, NQ, QC], BF, name="at_qc")
                    for h in range(NQ):
                        kvh = h // (NQ // NKV)
                        attn_ps = ap_.tile([128, QC], F32, name="attn_ps")
                        den_ps = dp.tile([1, QC], F32, name="den_ps")
                        for kc in range(KT):
                            s_ps = sp.tile([128, QC], F32, name="s_ps")
                            nc.tensor.matmul(
                                s_ps, k_sb[:, kvh, kc * 128:(kc + 1) * 128],
                                q_sb[:, h, qsl], start=True, stop=True)
                            e_t = ep.tile([128, QC], BF, name="e_t")
                            nc.scalar.activation(
                                out=e_t, in_=s_ps,
                                func=mybir.ActivationFunctionType.Exp, scale=scale)
                            nc.tensor.matmul(
                                attn_ps, v_sb[:, kc, kvh * 128:(kvh + 1) * 128], e_t,
                                start=(kc == 0), stop=(kc == KT - 1),
                                skip_group_check=True)
                            nc.tensor.matmul(
                                den_ps, ones_sb, e_t,
                                start=(kc == 0), stop=(kc == KT - 1),
                                skip_group_check=True)
                        rec = rb.tile([1, QC], F32, name="rec")
                        nc.vector.reciprocal(out=rec, in_=den_ps)
                        rec_bc = rb.tile([128, QC], F32, name="rec_bc")
                        nc.gpsimd.partition_broadcast(rec_bc, rec)
                        nc.vector.tensor_mul(at_qc[:, h, :], attn_ps, rec_bc)

                    # output projection for this q-chunk
                    for n in range(NO):
                        wos = wob.tile([128, NQ, 512], BF, name="wo_slab")
                        nc.sync.dma_start(
                            out=wos, in_=wo[:, n].rearrange("c p m -> p c m"))
                        o_t = ob.tile([128, QC // 128, 512], F32, name="o_t")
                        for tt in range(QC // 128):
                            ps_o = op.tile([128, 512], F32, name="ps_o")
                            for c in range(NQ):
                                nc.tensor.matmul(
                                    ps_o, at_qc[:, c, tt * 128:(tt + 1) * 128],
                                    wos[:, c, :],
                                    start=(c == 0), stop=(c == NQ - 1))
                            nc.vector.tensor_copy(o_t[:, tt, :], ps_o)
                        nc.sync.dma_start(
                            out=o[qsl, n * 512:(n + 1) * 512].rearrange(
                                "(t p) m -> p t m", p=128),
                            in_=o_t)
    nc.compile()
    return nc


def _rope_tables(position_ids_b, S_):
    """cos/sin tables in [d=128, s] layout, sin sign-folded for the half-swap."""
    pos = position_ids_b.astype(np.float32)
    inv_freq = (1.0 / (ROPE_THETA ** (np.arange(0, HEAD_DIM, 2, dtype=np.float32)
                                      / HEAD_DIM))).astype(np.float32)
    freqs = pos[:, None] * inv_freq[None, :]          # [s, 64]
    emb = np.concatenate([freqs, freqs], axis=1)      # [s, 128]
    cos = np.cos(emb).T.copy()                        # [128, s]
    sin = np.sin(emb).T.copy()
    sin[:64] *= -1.0                                  # sign-fold for swap rope
    return cos.astype(BF_NP), sin.astype(BF_NP)


def _prep_core_inputs(hidden_states, position_ids, Wq, Wk, Wv, Wo):
    rmat = np.zeros((128, 128), dtype=np.float32)
    for i in range(128):
        rmat[i, (i + 64) % 128] = 1.0
    rmat = rmat.astype(BF_NP)

    HC = HIDDEN // 128
    in_maps = []
    for t in range(TP):
        fq = slice(1024 * t, 1024 * (t + 1))
        fkv = slice(256 * t, 256 * (t + 1))
        wq_t = np.ascontiguousarray(
            Wq[:, fq].reshape(HC, 128, 8, 128).transpose(0, 2, 1, 3)).astype(BF_NP)
        wk_t = np.ascontiguousarray(
            Wk[:, fkv].reshape(HC, 128, 2, 128).transpose(0, 2, 1, 3)).astype(BF_NP)
        wv_t = np.ascontiguousarray(Wv[:, fkv].reshape(HC, 128, 256)).astype(BF_NP)
        wo_t = np.ascontiguousarray(
            Wo[fq, :].reshape(8, 128, 8, 512).transpose(0, 2, 1, 3)).astype(BF_NP)
        for b in range(B):
            xt = np.ascontiguousarray(hidden_states[b].T).astype(BF_NP)
            cos, sin = _rope_tables(position_ids[b], S)
            in_maps.append({"xt": xt, "wq": wq_t, "wk": wk_t, "wv": wv_t,
                            "wo": wo_t, "cos": cos, "sin": sin, "rmat": rmat})
    return in_maps


_NC_CACHE = {}


def kernel(hidden_states, position_ids, Wq, Wk, Wv, Wo):
    if "nc" not in _NC_CACHE:
        _NC_CACHE["nc"] = build_nc(FULL_CFG)
    nc = _NC_CACHE["nc"]
    in_maps = _prep_core_inputs(np.asarray(hidden_states), np.asarray(position_ids),
                                np.asarray(Wq), np.asarray(Wk),
                                np.asarray(Wv), np.asarray(Wo))
    res = run_bass_kernel_spmd(nc, in_maps, core_ids=list(range(8)))
    out = np.zeros((B, S, HIDDEN), dtype=np.float32)
    for t in range(TP):
        for b in range(B):
            out[b] += res.results[t * B + b]["o"]
    return out



# revision 8
# speedup vs baseline: 1.1669x; 1.1669x over previous
"""GQA attention kernel for Trainium2, 8-core SPMD.  v2: software-pipelined.

Sharding: tensor-parallel=4 over kv-head pairs x data-parallel=2 over batch.
Each core: one batch, 8 q-heads, 2 kv-heads, full 2048-token sequence.

Structure (per core):
  Stage A: K proj (+rope), V proj, Q proj for token block 0 (+rope).
  Stage B: per q-chunk qc: attention for 8 heads, software-pipelined:
    iter h: scores(h) [2 matmuls/kc-pair] -> exp pair [ACT, [128,1024] -> bf16]
            attnV(h-1) accumulate, then DVE in-place tree-fold of e(h-1)
            for the softmax denominator; den broadcast via ones[128x128]
            matmul; reciprocal_approx_fast; normalize -> at tile.
    Tensor idle slots are filled from a queue of "filler" units:
    Q projection (+rope) for block qc+1 and O projection for qc-1.
  Softmax denominator: DVE tree-fold (bf16) + one [128x128]-ones matmul
  per (head, qc) producing a broadcast den in PSUM (no gpsimd broadcast,
  no [1,512] DVE reciprocal).
All matmuls bf16 inputs / fp32 PSUM accumulation.
"""
import numpy as np
import ml_dtypes
from collections import deque

import concourse.bacc as bacc
import concourse.bass as bass
import concourse.tile as tile
from concourse import mybir
from concourse.bass_utils import run_bass_kernel_spmd

BF = mybir.dt.bfloat16
F32 = mybir.dt.float32
BF_NP = np.dtype(ml_dtypes.bfloat16)

# full-problem constants
B, S, HIDDEN = 2, 2048, 4096
NUM_HEADS, NUM_KV_HEADS, HEAD_DIM = 32, 8, 128
GROUPS = NUM_HEADS // NUM_KV_HEADS
ROPE_THETA = 10000.0
TP = 4  # shards over kv-head pairs

FULL_CFG = dict(S=2048, HID=4096, NQ=8, NKV=2, SB=512, QC=512)


def build_nc(cfg):
    S_, HID, NQ, NKV, SB, QC = (cfg[k] for k in ("S", "HID", "NQ", "NKV", "SB", "QC"))
    D = 128
    HC = HID // 128          # hidden chunks (contraction tiles)
    NB = S_ // SB            # token blocks (also q-chunks)
    KT = S_ // 128           # k-token tiles
    NP = KT // 2             # kc pairs
    DV = NKV * 128           # local v width
    NO = HID // 512          # O-proj output chunks
    GRP = NQ // NKV
    scale = 1.0 / np.sqrt(128.0)

    nc = bacc.Bacc("TRN2", target_bir_lowering=False, debug=False)
    xt = nc.dram_tensor("xt", (HID, S_), BF, kind="ExternalInput").ap()
    wq = nc.dram_tensor("wq", (HC, NQ, 128, 128), BF, kind="ExternalInput").ap()
    wk = nc.dram_tensor("wk", (HC, NKV, 128, 128), BF, kind="ExternalInput").ap()
    wv = nc.dram_tensor("wv", (HC, 128, DV), BF, kind="ExternalInput").ap()
    wo = nc.dram_tensor("wo", (NQ, NO, 128, 512), BF, kind="ExternalInput").ap()
    cosd = nc.dram_tensor("cos", (128, S_), BF, kind="ExternalInput").ap()
    sind = nc.dram_tensor("sin", (128, S_), BF, kind="ExternalInput").ap()
    rmatd = nc.dram_tensor("rmat", (128, 128), BF, kind="ExternalInput").ap()
    o = nc.dram_tensor("o", (S_, HID), F32, kind="ExternalOutput").ap()

    xt_r = xt.rearrange("(c p) s -> p c s", p=128)

    with tile.TileContext(nc) as tc:
        with tc.tile_pool(name="cons", bufs=1) as cons, \
             tc.tile_pool(name="big", bufs=1) as big:
            cos_sb = cons.tile([128, S_], BF, name="cos_sb")
            sin_sb = cons.tile([128, S_], BF, name="sin_sb")
            r_sb = cons.tile([128, 128], BF, name="r_sb")
            ones_sb = cons.tile([128, 128], BF, name="ones_sb")
            nc.sync.dma_start(out=cos_sb, in_=cosd)
            nc.sync.dma_start(out=sin_sb, in_=sind)
            nc.sync.dma_start(out=r_sb, in_=rmatd)
            nc.vector.memset(ones_sb, 1.0)

            q_sb = big.tile([128, NQ, S_], BF, name="q_sb")
            k_sb = big.tile([128, NKV, S_], BF, name="k_sb")
            v_sb = big.tile([128, KT, DV], BF, name="v_sb")

            # ---------------- stage A: K, V, Q(block0) projections ----------
            with tc.tile_pool(name="xpA", bufs=2) as xpA, \
                 tc.tile_pool(name="wpA", bufs=2) as wpA, \
                 tc.tile_pool(name="wvp", bufs=1) as wvp, \
                 tc.tile_pool(name="qbfA", bufs=2) as qbfA, \
                 tc.tile_pool(name="tpA", bufs=2) as tpA, \
                 tc.tile_pool(name="ppA", bufs=2, space="PSUM") as ppA, \
                 tc.tile_pool(name="rotA", bufs=2, space="PSUM") as rotA:
                wv_sb = wvp.tile([128, HC, DV], BF, name="wv_sb")
                nc.sync.dma_start(out=wv_sb, in_=wv.rearrange("c p v -> p c v"))

                def rope_combine(ps, rot_pool, qbf_pool, t_pool, dst_ap, ssl):
                    """dst = ps*cos + (R@ps)*sin_folded over token slice ssl."""
                    qbf = qbf_pool.tile([128, SB], BF, name="rope_bf")
                    nc.scalar.activation(out=qbf, in_=ps,
                                         func=mybir.ActivationFunctionType.Copy)
                    rot = rot_pool.tile([128, SB], F32, name="rot_ps")
                    nc.tensor.matmul(rot, r_sb, qbf, start=True, stop=True)
                    t1 = t_pool.tile([128, SB], F32, name="rope_t1")
                    t2 = t_pool.tile([128, SB], F32, name="rope_t2")
                    nc.vector.tensor_mul(t1, ps, cos_sb[:, ssl])
                    nc.vector.tensor_mul(t2, rot, sin_sb[:, ssl])
                    nc.vector.tensor_add(dst_ap, t1, t2)

                for sb_i in range(NB):
                    ssl = slice(sb_i * SB, (sb_i + 1) * SB)
                    xt_t = xpA.tile([128, HC, SB], BF, name="xt_tA")
                    nc.sync.dma_start(out=xt_t, in_=xt_r[:, :, ssl])

                    heads = [("k", kvh) for kvh in range(NKV)]
                    if sb_i == 0:
                        heads += [("q", h) for h in range(NQ)]
                    for which, h in heads:
                        wten, dst = (wk, k_sb) if which == "k" else (wq, q_sb)
                        wslab = wpA.tile([128, HC, 128], BF, name="w_slabA")
                        nc.sync.dma_start(
                            out=wslab, in_=wten[:, h].rearrange("c p m -> p c m"))
                        ps = ppA.tile([128, SB], F32, name="ps_projA")
                        for c in range(HC):
                            nc.tensor.matmul(ps, wslab[:, c, :], xt_t[:, c, :],
                                             start=(c == 0), stop=(c == HC - 1))
                        rope_combine(ps, rotA, qbfA, tpA, dst[:, h, ssl], ssl)

                    # V projection (natural [tok, d] layout)
                    for tt in range(SB // 128):
                        ps = ppA.tile([128, DV], F32, name="ps_vA")
                        for c in range(HC):
                            nc.tensor.matmul(ps, xt_t[:, c, tt * 128:(tt + 1) * 128],
                                             wv_sb[:, c, :],
                                             start=(c == 0), stop=(c == HC - 1))
                        nc.scalar.activation(out=v_sb[:, sb_i * (SB // 128) + tt, :],
                                             in_=ps,
                                             func=mybir.ActivationFunctionType.Copy)

            # ------------- stage B: attention + Q proj + O proj pipelined ----
            with tc.tile_pool(name="xpB", bufs=1) as xpB, \
                 tc.tile_pool(name="wpB", bufs=2) as wpB, \
                 tc.tile_pool(name="wob", bufs=2) as wob, \
                 tc.tile_pool(name="ep", bufs=2) as ep, \
                 tc.tile_pool(name="atp", bufs=2) as atp, \
                 tc.tile_pool(name="esp", bufs=2) as esp, \
                 tc.tile_pool(name="rcp", bufs=2) as rcp, \
                 tc.tile_pool(name="qbfB", bufs=2) as qbfB, \
                 tc.tile_pool(name="tpB", bufs=2) as tpB, \
                 tc.tile_pool(name="otp", bufs=2) as otp, \
                 tc.tile_pool(name="sp", bufs=1, space="PSUM") as sp, \
                 tc.tile_pool(name="ap_", bufs=2, space="PSUM") as ap_, \
                 tc.tile_pool(name="qr", bufs=2, space="PSUM") as qr, \
                 tc.tile_pool(name="dp", bufs=2, space="PSUM") as dp:

                filler = deque()

                def pump(n):
                    """Emit ~n matmuls worth of filler work."""
                    while n > 0 and filler:
                        try:
                            n -= next(filler[0])
                        except StopIteration:
                            filler.popleft()

                def drain_filler():
                    while filler:
                        try:
                            next(filler[0])
                        except StopIteration:
                            filler.popleft()

                xt_cur = [None]

                def xload_unit(b):
                    ssl = slice(b * SB, (b + 1) * SB)
                    xt_t = xpB.tile([128, HC, SB], BF, name="xt_tB")
                    nc.sync.dma_start(out=xt_t, in_=xt_r[:, :, ssl])
                    xt_cur[0] = xt_t
                    yield 0

                def qproj_unit(b, h):
                    ssl = slice(b * SB, (b + 1) * SB)
                    wslab = wpB.tile([128, HC, 128], BF, name="w_slabB")
                    nc.sync.dma_start(
                        out=wslab, in_=wq[:, h].rearrange("c p m -> p c m"))
                    yield 0
                    xt_t = xt_cur[0]
                    ps = qr.tile([128, SB], F32, name="qr_ps")
                    for c0 in range(0, HC, 8):
                        for c in range(c0, c0 + 8):
                            nc.tensor.matmul(ps, wslab[:, c, :], xt_t[:, c, :],
                                             start=(c == 0), stop=(c == HC - 1),
                                             skip_group_check=True)
                        yield 8
                    # rope
                    qbf = qbfB.tile([128, SB], BF, name="rope_bfB")
                    nc.scalar.activation(out=qbf, in_=ps,
                                         func=mybir.ActivationFunctionType.Copy)
                    rot = qr.tile([128, SB], F32, name="qr_ps")
                    nc.tensor.matmul(rot, r_sb, qbf, start=True, stop=True)
                    t1 = tpB.tile([128, SB], F32, name="rope_t1B")
                    t2 = tpB.tile([128, SB], F32, name="rope_t2B")
                    nc.vector.tensor_mul(t1, ps, cos_sb[:, ssl])
                    nc.vector.tensor_mul(t2, rot, sin_sb[:, ssl])
                    nc.vector.tensor_add(q_sb[:, h, ssl], t1, t2)
                    yield 1

                def oproj_unit(qc, n, at_t):
                    qsl0 = qc * QC
                    wos = wob.tile([128, NQ, 512], BF, name="wo_slab")
                    nc.sync.dma_start(
                        out=wos, in_=wo[:, n].rearrange("c p m -> p c m"))
                    yield 0
                    for tt in range(QC // 128):
                        ps_o = dp.tile([128, 512], F32, name="dp_ps")
                        for c in range(NQ):
                            nc.tensor.matmul(
                                ps_o, at_t[:, c, tt * 128:(tt + 1) * 128],
                                wos[:, c, :],
                                start=(c == 0), stop=(c == NQ - 1),
                                skip_group_check=True)
                        yield NQ
                        o_t = otp.tile([128, 512], F32, name="o_t")
                        nc.scalar.activation(out=o_t, in_=ps_o,
                                             func=mybir.ActivationFunctionType.Copy)
                        nc.sync.dma_start(
                            out=o[qsl0 + tt * 128: qsl0 + (tt + 1) * 128,
                                  n * 512:(n + 1) * 512],
                            in_=o_t)
                        yield 0

                # pipeline state per in-flight head
                e_tiles = {}      # h -> e tile [128, KT, 512]
                aps_tiles = {}    # h -> attn psum [128, QC]
                esum_tiles = {}   # h -> [128, 512] bf16

                for qc in range(NB):
                    qsl = slice(qc * QC, (qc + 1) * QC)
                    at_t = atp.tile([128, NQ, QC], BF, name="at_t")
                    if qc < NB - 1:
                        filler.append(xload_unit(qc + 1))
                        for h in range(NQ):
                            filler.append(qproj_unit(qc + 1, h))

                    for it in range(NQ + 2):
                        h_s = it if it < NQ else None          # scores/exp head
                        h_a = it - 1 if 1 <= it <= NQ else None  # attnV/tree head
                        h_d = it - 2 if it >= 2 else None      # den/norm head

                        if h_a is not None:
                            kvh = h_a // GRP
                            aps = ap_.tile([128, QC], F32, name="attn_ps")
                            aps_tiles[h_a] = aps
                        if h_s is not None:
                            kvh_s = h_s // GRP
                            e_t = ep.tile([128, KT, 512], BF, name="e_t")
                            e_tiles[h_s] = e_t
                        for p in range(NP if h_s is not None else 0):
                            s_pair = sp.tile([128, 2, 512], F32, name="s_pair")
                            for j in range(2):
                                kc = 2 * p + j
                                nc.tensor.matmul(
                                    s_pair[:, j, :],
                                    k_sb[:, kvh_s, kc * 128:(kc + 1) * 128],
                                    q_sb[:, h_s, qsl], start=True, stop=True)
                            nc.scalar.activation(
                                out=e_tiles[h_s][:, 2 * p:2 * p + 2, :],
                                in_=s_pair,
                                func=mybir.ActivationFunctionType.Exp, scale=scale)
                            if h_a is not None:
                                for j in range(2):
                                    kc = 2 * p + j
                                    nc.tensor.matmul(
                                        aps_tiles[h_a],
                                        v_sb[:, kc, kvh * 128:(kvh + 1) * 128],
                                        e_tiles[h_a][:, kc, :],
                                        start=(kc == 0), stop=(kc == KT - 1),
                                        skip_group_check=True)
                            pump(4)
                        if h_s is None and h_a is not None:
                            # drain iteration: attnV without concurrent scores
                            for p in range(NP):
                                for j in range(2):
                                    kc = 2 * p + j
                                    nc.tensor.matmul(
                                        aps_tiles[h_a],
                                        v_sb[:, kc, kvh * 128:(kvh + 1) * 128],
                                        e_tiles[h_a][:, kc, :],
                                        start=(kc == 0), stop=(kc == KT - 1),
                                        skip_group_check=True)
                                pump(4)

                        if h_a is not None:
                            # tree-fold e(h_a) in place for the denominator
                            e_t = e_tiles[h_a]
                            nc.vector.tensor_add(e_t[:, 0:4, :], e_t[:, 0:4, :],
                                                 e_t[:, 4:8, :])
                            nc.vector.tensor_add(e_t[:, 8:12, :], e_t[:, 8:12, :],
                                                 e_t[:, 12:16, :])
                            nc.vector.tensor_add(e_t[:, 0:2, :], e_t[:, 0:2, :],
                                                 e_t[:, 2:4, :])
                            nc.vector.tensor_add(e_t[:, 8:10, :], e_t[:, 8:10, :],
                                                 e_t[:, 10:12, :])
                            nc.vector.tensor_add(e_t[:, 0:2, :], e_t[:, 0:2, :],
                                                 e_t[:, 8:10, :])
                            esum = esp.tile([128, 512], BF, name="esum")
                            nc.vector.tensor_add(esum, e_t[:, 0, :], e_t[:, 1, :])
                            esum_tiles[h_a] = esum

                        if h_d is not None:
                            den_bc = dp.tile([128, 512], F32, name="dp_ps")
                            nc.tensor.matmul(den_bc, ones_sb, esum_tiles.pop(h_d),
                                             start=True, stop=True)
                            rec = rcp.tile([128, 512], F32, name="rec")
                            nc.vector.reciprocal_approx_fast(out=rec, in_=den_bc)
                            nc.vector.tensor_mul(at_t[:, h_d, :],
                                                 aps_tiles.pop(h_d), rec)
                            del e_tiles[h_d]
                            pump(2)

                    # O projection for this qc becomes filler for the next
                    for n in range(NO):
                        filler.append(oproj_unit(qc, n, at_t))
                    if qc == NB - 1:
                        drain_filler()
                drain_filler()
    nc.compile()
    return nc


def _rope_tables(position_ids_b, S_):
    """cos/sin tables in [d=128, s] layout, sin sign-folded for the half-swap."""
    pos = position_ids_b.astype(np.float32)
    inv_freq = (1.0 / (ROPE_THETA ** (np.arange(0, HEAD_DIM, 2, dtype=np.float32)
                                      / HEAD_DIM))).astype(np.float32)
    freqs = pos[:, None] * inv_freq[None, :]          # [s, 64]
    emb = np.concatenate([freqs, freqs], axis=1)      # [s, 128]
    cos = np.cos(emb).T.copy()                        # [128, s]
    sin = np.sin(emb).T.copy()
    sin[:64] *= -1.0                                  # sign-fold for swap rope
    return cos.astype(BF_NP), sin.astype(BF_NP)


def _prep_core_inputs(hidden_states, position_ids, Wq, Wk, Wv, Wo):
    rmat = np.zeros((128, 128), dtype=np.float32)
    for i in range(128):
        rmat[i, (i + 64) % 128] = 1.0
    rmat = rmat.astype(BF_NP)

    HC = HIDDEN // 128
    in_maps = []
    for t in range(TP):
        fq = slice(1024 * t, 1024 * (t + 1))
        fkv = slice(256 * t, 256 * (t + 1))
        wq_t = np.ascontiguousarray(
            Wq[:, fq].reshape(HC, 128, 8, 128).transpose(0, 2, 1, 3)).astype(BF_NP)
        wk_t = np.ascontiguousarray(
            Wk[:, fkv].reshape(HC, 128, 2, 128).transpose(0, 2, 1, 3)).astype(BF_NP)
        wv_t = np.ascontiguousarray(Wv[:, fkv].reshape(HC, 128, 256)).astype(BF_NP)
        wo_t = np.ascontiguousarray(
            Wo[fq, :].reshape(8, 128, 8, 512).transpose(0, 2, 1, 3)).astype(BF_NP)
        for b in range(B):
            xt = np.ascontiguousarray(hidden_states[b].T).astype(BF_NP)
            cos, sin = _rope_tables(position_ids[b], S)
            in_maps.append({"xt": xt, "wq": wq_t, "wk": wk_t, "wv": wv_t,
                            "wo": wo_t, "cos": cos, "sin": sin, "rmat": rmat})
    return in_maps


_NC_CACHE = {}


def kernel(hidden_states, position_ids, Wq, Wk, Wv, Wo):
    if "nc" not in _NC_CACHE:
        _NC_CACHE["nc"] = build_nc(FULL_CFG)
    nc = _NC_CACHE["nc"]
    in_maps = _prep_core_inputs(np.asarray(hidden_states), np.asarray(position_ids),
                                np.asarray(Wq), np.asarray(Wk),
                                np.asarray(Wv), np.asarray(Wo))
    res = run_bass_kernel_spmd(nc, in_maps, core_ids=list(range(8)))
    out = np.zeros((B, S, HIDDEN), dtype=np.float32)
    for t in range(TP):
        for b in range(B):
            out[b] += res.results[t * B + b]["o"]
    return out


# revision 19
# speedup vs baseline: 1.2324x; 1.0561x over previous
"""GQA attention kernel for Trainium2, 8-core SPMD.  v2: software-pipelined.

Sharding: tensor-parallel=4 over kv-head pairs x data-parallel=2 over batch.
Each core: one batch, 8 q-heads, 2 kv-heads, full 2048-token sequence.

Structure (per core):
  Stage A: K proj (+rope), V proj, Q proj for token block 0 (+rope).
  Stage B: per q-chunk qc: attention for 8 heads, software-pipelined:
    iter h: scores(h) [2 matmuls/kc-pair] -> exp pair [ACT, [128,1024] -> bf16]
            attnV(h-1) accumulate, then DVE in-place tree-fold of e(h-1)
            for the softmax denominator; den broadcast via ones[128x128]
            matmul; reciprocal_approx_fast; normalize -> at tile.
    Tensor idle slots are filled from a queue of "filler" units:
    Q projection (+rope) for block qc+1 and O projection for qc-1.
  Softmax denominator: DVE tree-fold (bf16) + one [128x128]-ones matmul
  per (head, qc) producing a broadcast den in PSUM (no gpsimd broadcast,
  no [1,512] DVE reciprocal).
All matmuls bf16 inputs / fp32 PSUM accumulation.
"""
import numpy as np
import ml_dtypes
from collections import deque

import concourse.bacc as bacc
import concourse.bass as bass
import concourse.tile as tile
from concourse import mybir
from concourse.bass_utils import run_bass_kernel_spmd

BF = mybir.dt.bfloat16
F32 = mybir.dt.float32
BF_NP = np.dtype(ml_dtypes.bfloat16)

# full-problem constants
B, S, HIDDEN = 2, 2048, 4096
NUM_HEADS, NUM_KV_HEADS, HEAD_DIM = 32, 8, 128
GROUPS = NUM_HEADS // NUM_KV_HEADS
ROPE_THETA = 10000.0
TP = 4  # shards over kv-head pairs

FULL_CFG = dict(S=2048, HID=4096, NQ=8, NKV=2, SB=512, QC=512)


def build_nc(cfg):
    S_, HID, NQ, NKV, SB, QC = (cfg[k] for k in ("S", "HID", "NQ", "NKV", "SB", "QC"))
    D = 128
    HC = HID // 128          # hidden chunks (contraction tiles)
    NB = S_ // SB            # token blocks (also q-chunks)
    KT = S_ // 128           # k-token tiles
    NP = KT // 2             # kc pairs
    DV = NKV * 128           # local v width
    NO = HID // 512          # O-proj output chunks
    GRP = NQ // NKV
    scale = 1.0 / np.sqrt(128.0)

    nc = bacc.Bacc("TRN2", target_bir_lowering=False, debug=False)
    xt = nc.dram_tensor("xt", (HID, S_), BF, kind="ExternalInput").ap()
    wq = nc.dram_tensor("wq", (HC, NQ, 128, 128), BF, kind="ExternalInput").ap()
    wk = nc.dram_tensor("wk", (HC, NKV, 128, 128), BF, kind="ExternalInput").ap()
    wv = nc.dram_tensor("wv", (HC, 128, DV), BF, kind="ExternalInput").ap()
    wo = nc.dram_tensor("wo", (NQ, NO, 128, 512), BF, kind="ExternalInput").ap()
    cosd = nc.dram_tensor("cos", (128, S_), BF, kind="ExternalInput").ap()
    sind = nc.dram_tensor("sin", (128, S_), BF, kind="ExternalInput").ap()
    rmatd = nc.dram_tensor("rmat", (128, 128), BF, kind="ExternalInput").ap()
    o = nc.dram_tensor("o", (S_, HID), F32, kind="ExternalOutput").ap()

    xt_r = xt.rearrange("(c p) s -> p c s", p=128)

    with tile.TileContext(nc) as tc:
        with tc.tile_pool(name="cons", bufs=1) as cons, \
             tc.tile_pool(name="big", bufs=1) as big:
            cos_sb = cons.tile([128, S_], BF, name="cos_sb")
            sin_sb = cons.tile([128, S_], BF, name="sin_sb")
            r_sb = cons.tile([128, 128], BF, name="r_sb")
            ones_sb = cons.tile([128, 128], BF, name="ones_sb")
            nc.sync.dma_start(out=cos_sb, in_=cosd)
            nc.sync.dma_start(out=sin_sb, in_=sind)
            nc.sync.dma_start(out=r_sb, in_=rmatd)
            nc.vector.memset(ones_sb, 1.0)

            q_sb = big.tile([128, NQ, S_], BF, name="q_sb")
            k_sb = big.tile([128, NKV, S_], BF, name="k_sb")
            v_sb = big.tile([128, KT, DV], BF, name="v_sb")

            # ---------------- stage A: K, V, Q(block0) projections ----------
            with tc.tile_pool(name="xpA", bufs=2) as xpA, \
                 tc.tile_pool(name="wpA", bufs=2) as wpA, \
                 tc.tile_pool(name="wvp", bufs=1) as wvp, \
                 tc.tile_pool(name="qbfA", bufs=2) as qbfA, \
                 tc.tile_pool(name="tpA", bufs=2) as tpA, \
                 tc.tile_pool(name="ppA", bufs=2, space="PSUM") as ppA, \
                 tc.tile_pool(name="rotA", bufs=2, space="PSUM") as rotA:
                wv_sb = wvp.tile([128, HC, DV], BF, name="wv_sb")
                nc.sync.dma_start(out=wv_sb, in_=wv.rearrange("c p v -> p c v"))

                def rope_combine(ps, rot_pool, qbf_pool, t_pool, dst_ap, ssl):
                    """dst = ps*cos + (R@ps)*sin_folded over token slice ssl."""
                    qbf = qbf_pool.tile([128, SB], BF, name="rope_bf")
                    nc.scalar.activation(out=qbf, in_=ps,
                                         func=mybir.ActivationFunctionType.Copy)
                    rot = rot_pool.tile([128, SB], F32, name="rot_ps")
                    nc.tensor.matmul(rot, r_sb, qbf, start=True, stop=True)
                    t1 = t_pool.tile([128, SB], F32, name="rope_t1")
                    t2 = t_pool.tile([128, SB], F32, name="rope_t2")
                    nc.vector.tensor_mul(t1, ps, cos_sb[:, ssl])
                    nc.vector.tensor_mul(t2, rot, sin_sb[:, ssl])
                    nc.vector.tensor_add(dst_ap, t1, t2)

                for sb_i in range(NB):
                    ssl = slice(sb_i * SB, (sb_i + 1) * SB)
                    xt_t = xpA.tile([128, HC, SB], BF, name="xt_tA")
                    # split the load so the first matmuls start after 1/4 of it
                    for c0 in range(0, HC, 8):
                        nc.sync.dma_start(out=xt_t[:, c0:c0 + 8, :],
                                          in_=xt_r[:, c0:c0 + 8, ssl])

                    heads = [("k", kvh) for kvh in range(NKV)]
                    if sb_i == 0:
                        heads += [("q", h) for h in range(NQ)]
                    for which, h in heads:
                        wten, dst = (wk, k_sb) if which == "k" else (wq, q_sb)
                        wslab = wpA.tile([128, HC, 128], BF, name="w_slabA")
                        nc.sync.dma_start(
                            out=wslab, in_=wten[:, h].rearrange("c p m -> p c m"))
                        ps = ppA.tile([128, SB], F32, name="ps_projA")
                        for c in range(HC):
                            nc.tensor.matmul(ps, wslab[:, c, :], xt_t[:, c, :],
                                             start=(c == 0), stop=(c == HC - 1))
                        rope_combine(ps, rotA, qbfA, tpA, dst[:, h, ssl], ssl)

                    # V projection (natural [tok, d] layout)
                    for tt in range(SB // 128):
                        ps = ppA.tile([128, DV], F32, name="ps_vA")
                        for c in range(HC):
                            nc.tensor.matmul(ps, xt_t[:, c, tt * 128:(tt + 1) * 128],
                                             wv_sb[:, c, :],
                                             start=(c == 0), stop=(c == HC - 1))
                        nc.scalar.activation(out=v_sb[:, sb_i * (SB // 128) + tt, :],
                                             in_=ps,
                                             func=mybir.ActivationFunctionType.Copy)

            # ------------- stage B: attention + Q proj + O proj pipelined ----
            with tc.tile_pool(name="xpB", bufs=1) as xpB, \
                 tc.tile_pool(name="wpB", bufs=2) as wpB, \
                 tc.tile_pool(name="wob", bufs=2) as wob, \
                 tc.tile_pool(name="ep", bufs=2) as ep, \
                 tc.tile_pool(name="atp", bufs=2) as atp, \
                 tc.tile_pool(name="esp", bufs=2) as esp, \
                 tc.tile_pool(name="rcp", bufs=2) as rcp, \
                 tc.tile_pool(name="qbfB", bufs=2) as qbfB, \
                 tc.tile_pool(name="tpB", bufs=2) as tpB, \
                 tc.tile_pool(name="otp", bufs=2) as otp, \
                 tc.tile_pool(name="sp", bufs=1, space="PSUM") as sp, \
                 tc.tile_pool(name="ap_", bufs=2, space="PSUM") as ap_, \
                 tc.tile_pool(name="qr", bufs=2, space="PSUM") as qr, \
                 tc.tile_pool(name="dp", bufs=2, space="PSUM") as dp:

                filler = deque()
                iter_budget = [10 ** 9]

                def pump(n):
                    """Emit ~n matmuls worth of filler work (iter-budgeted)."""
                    n = min(n, iter_budget[0])
                    while n > 0 and filler:
                        try:
                            k = next(filler[0])
                            n -= k
                            iter_budget[0] -= k
                        except StopIteration:
                            filler.popleft()

                def drain_filler():
                    while filler:
                        try:
                            next(filler[0])
                        except StopIteration:
                            filler.popleft()

                xt_cur = [None]

                def xload_unit(b):
                    ssl = slice(b * SB, (b + 1) * SB)
                    xt_t = xpB.tile([128, HC, SB], BF, name="xt_tB")
                    nc.sync.dma_start(out=xt_t, in_=xt_r[:, :, ssl])
                    xt_cur[0] = xt_t
                    yield 0

                def qproj_unit(b, h):
                    ssl = slice(b * SB, (b + 1) * SB)
                    wslab = wpB.tile([128, HC, 128], BF, name="w_slabB")
                    nc.sync.dma_start(
                        out=wslab, in_=wq[:, h].rearrange("c p m -> p c m"))
                    yield 0
                    xt_t = xt_cur[0]
                    ps = qr.tile([128, SB], F32, name="qr_ps")
                    for c0 in range(0, HC, 8):
                        for c in range(c0, c0 + 8):
                            nc.tensor.matmul(ps, wslab[:, c, :], xt_t[:, c, :],
                                             start=(c == 0), stop=(c == HC - 1),
                                             skip_group_check=True)
                        yield 8
                    # rope (t1 emitted before rot-mm so the qps slot frees
                    # without depending on later tensor work)
                    qbf = qbfB.tile([128, SB], BF, name="rope_bfB")
                    nc.scalar.activation(out=qbf, in_=ps,
                                         func=mybir.ActivationFunctionType.Copy)
                    t1 = tpB.tile([128, SB], F32, name="rope_t1B")
                    nc.vector.tensor_mul(t1, ps, cos_sb[:, ssl])
                    yield 4  # let ACT drain qbf before the rot matmul needs it
                    rot = qr.tile([128, SB], F32, name="qr_ps")
                    nc.tensor.matmul(rot, r_sb, qbf, start=True, stop=True)
                    t2 = tpB.tile([128, SB], F32, name="rope_t2B")
                    nc.vector.tensor_mul(t2, rot, sin_sb[:, ssl])
                    nc.vector.tensor_add(q_sb[:, h, ssl], t1, t2)
                    yield 1

                def oproj_unit(qc, n, at_t):
                    qsl0 = qc * QC
                    wos = wob.tile([128, NQ, 512], BF, name="wo_slab")
                    nc.sync.dma_start(
                        out=wos, in_=wo[:, n].rearrange("c p m -> p c m"))
                    yield 0
                    for tt in range(QC // 128):
                        ps_o = dp.tile([128, 512], F32, name="dp_ps")
                        for c in range(NQ):
                            nc.tensor.matmul(
                                ps_o, at_t[:, c, tt * 128:(tt + 1) * 128],
                                wos[:, c, :],
                                start=(c == 0), stop=(c == NQ - 1),
                                skip_group_check=True)
                        yield NQ
                        o_t = otp.tile([128, 512], F32, name="o_t")
                        if tt % 2 == 0:
                            nc.scalar.activation(
                                out=o_t, in_=ps_o,
                                func=mybir.ActivationFunctionType.Copy)
                        else:
                            nc.vector.tensor_copy(o_t, ps_o)
                        nc.sync.dma_start(
                            out=o[qsl0 + tt * 128: qsl0 + (tt + 1) * 128,
                                  n * 512:(n + 1) * 512],
                            in_=o_t)
                        yield 0

                # pipeline state per in-flight head
                e_tiles = {}      # h -> e tile [128, KT, 512]
                aps_tiles = {}    # h -> attn psum [128, QC]
                esum_tiles = {}   # h -> [128, 512] bf16

                budget_tbl = [28, 55, 55, 27]
                for qc in range(NB):
                    qsl = slice(qc * QC, (qc + 1) * QC)
                    at_t = atp.tile([128, NQ, QC], BF, name="at_t")
                    if qc < NB - 1:
                        filler.append(xload_unit(qc + 1))
                        for h in range(NQ):
                            filler.append(qproj_unit(qc + 1, h))

                    for it in range(NQ + 2):
                        iter_budget[0] = budget_tbl[qc]
                        h_s = it if it < NQ else None          # scores/exp head
                        h_a = it - 1 if 1 <= it <= NQ else None  # attnV/tree head
                        h_d = it - 2 if it >= 2 else None      # den/norm head

                        if h_a is not None:
                            kvh = h_a // GRP
                            aps = ap_.tile([128, QC], F32, name="attn_ps")
                            aps_tiles[h_a] = aps
                        if h_s is not None:
                            kvh_s = h_s // GRP
                            e_t = ep.tile([128, KT, 512], BF, name="e_t")
                            e_tiles[h_s] = e_t
                        for p in range(NP if h_s is not None else 0):
                            s_pair = sp.tile([128, 2, 512], F32, name="s_pair")
                            for j in range(2):
                                kc = 2 * p + j
                                nc.tensor.matmul(
                                    s_pair[:, j, :],
                                    k_sb[:, kvh_s, kc * 128:(kc + 1) * 128],
                                    q_sb[:, h_s, qsl], start=True, stop=True)
                            nc.scalar.activation(
                                out=e_tiles[h_s][:, 2 * p:2 * p + 2, :],
                                in_=s_pair,
                                func=mybir.ActivationFunctionType.Exp, scale=scale)
                            if h_a is not None:
                                for j in range(2):
                                    kc = 2 * p + j
                                    nc.tensor.matmul(
                                        aps_tiles[h_a],
                                        v_sb[:, kc, kvh * 128:(kvh + 1) * 128],
                                        e_tiles[h_a][:, kc, :],
                                        start=(kc == 0), stop=(kc == KT - 1),
                                        skip_group_check=True)
                            pump(8)
                        if h_s is None and h_a is not None:
                            # drain iteration: attnV without concurrent scores
                            for p in range(NP):
                                for j in range(2):
                                    kc = 2 * p + j
                                    nc.tensor.matmul(
                                        aps_tiles[h_a],
                                        v_sb[:, kc, kvh * 128:(kvh + 1) * 128],
                                        e_tiles[h_a][:, kc, :],
                                        start=(kc == 0), stop=(kc == KT - 1),
                                        skip_group_check=True)
                                pump(8)

                        if h_a is not None:
                            # tree-fold e(h_a) in place for the denominator
                            e_t = e_tiles[h_a]
                            nc.vector.tensor_add(e_t[:, 0:4, :], e_t[:, 0:4, :],
                                                 e_t[:, 4:8, :])
                            nc.vector.tensor_add(e_t[:, 8:12, :], e_t[:, 8:12, :],
                                                 e_t[:, 12:16, :])
                            nc.vector.tensor_add(e_t[:, 0:2, :], e_t[:, 0:2, :],
                                                 e_t[:, 2:4, :])
                            nc.vector.tensor_add(e_t[:, 8:10, :], e_t[:, 8:10, :],
                                                 e_t[:, 10:12, :])
                            nc.vector.tensor_add(e_t[:, 0:2, :], e_t[:, 0:2, :],
                                                 e_t[:, 8:10, :])
                            esum = esp.tile([128, 512], BF, name="esum")
                            nc.vector.tensor_add(esum, e_t[:, 0, :], e_t[:, 1, :])
                            esum_tiles[h_a] = esum

                        if h_d is not None:
                            den_bc = qr.tile([128, SB], F32, name="qr_ps")
                            nc.tensor.matmul(den_bc, ones_sb, esum_tiles.pop(h_d),
                                             start=True, stop=True)
                            rec = rcp.tile([128, 512], F32, name="rec")
                            nc.vector.reciprocal_approx_fast(out=rec, in_=den_bc)
                            nc.vector.tensor_mul(at_t[:, h_d, :],
                                                 aps_tiles.pop(h_d), rec)
                            del e_tiles[h_d]
                            pump(4)

                    # O projection for this qc becomes filler for the next
                    for n in range(NO):
                        filler.append(oproj_unit(qc, n, at_t))
                    if qc == NB - 1:
                        iter_budget[0] = 10 ** 9
                        drain_filler()
                iter_budget[0] = 10 ** 9
                drain_filler()
    nc.compile()
    return nc


def _rope_tables(position_ids_b, S_):
    """cos/sin tables in [d=128, s] layout, sin sign-folded for the half-swap."""
    pos = position_ids_b.astype(np.float32)
    inv_freq = (1.0 / (ROPE_THETA ** (np.arange(0, HEAD_DIM, 2, dtype=np.float32)
                                      / HEAD_DIM))).astype(np.float32)
    freqs = pos[:, None] * inv_freq[None, :]          # [s, 64]
    emb = np.concatenate([freqs, freqs], axis=1)      # [s, 128]
    cos = np.cos(emb).T.copy()                        # [128, s]
    sin = np.sin(emb).T.copy()
    sin[:64] *= -1.0                                  # sign-fold for swap rope
    return cos.astype(BF_NP), sin.astype(BF_NP)


def _prep_core_inputs(hidden_states, position_ids, Wq, Wk, Wv, Wo):
    rmat = np.zeros((128, 128), dtype=np.float32)
    for i in range(128):
        rmat[i, (i + 64) % 128] = 1.0
    rmat = rmat.astype(BF_NP)

    HC = HIDDEN // 128
    in_maps = []
    for t in range(TP):
        fq = slice(1024 * t, 1024 * (t + 1))
        fkv = slice(256 * t, 256 * (t + 1))
        wq_t = np.ascontiguousarray(
            Wq[:, fq].reshape(HC, 128, 8, 128).transpose(0, 2, 1, 3)).astype(BF_NP)
        wk_t = np.ascontiguousarray(
            Wk[:, fkv].reshape(HC, 128, 2, 128).transpose(0, 2, 1, 3)).astype(BF_NP)
        wv_t = np.ascontiguousarray(Wv[:, fkv].reshape(HC, 128, 256)).astype(BF_NP)
        wo_t = np.ascontiguousarray(
            Wo[fq, :].reshape(8, 128, 8, 512).transpose(0, 2, 1, 3)).astype(BF_NP)
        for b in range(B):
            xt = np.ascontiguousarray(hidden_states[b].T).astype(BF_NP)
            cos, sin = _rope_tables(position_ids[b], S)
            in_maps.append({"xt": xt, "wq": wq_t, "wk": wk_t, "wv": wv_t,
                            "wo": wo_t, "cos": cos, "sin": sin, "rmat": rmat})
    return in_maps


_NC_CACHE = {}


def kernel(hidden_states, position_ids, Wq, Wk, Wv, Wo):
    if "nc" not in _NC_CACHE:
        _NC_CACHE["nc"] = build_nc(FULL_CFG)
    nc = _NC_CACHE["nc"]
    in_maps = _prep_core_inputs(np.asarray(hidden_states), np.asarray(position_ids),
                                np.asarray(Wq), np.asarray(Wk),
                                np.asarray(Wv), np.asarray(Wo))
    res = run_bass_kernel_spmd(nc, in_maps, core_ids=list(range(8)))
    out = np.zeros((B, S, HIDDEN), dtype=np.float32)
    for t in range(TP):
        for b in range(B):
            out[b] += res.results[t * B + b]["o"]
    return out


# revision 23
# speedup vs baseline: 1.2722x; 1.0323x over previous
"""GQA attention kernel for Trainium2, 8-core SPMD.  v2: software-pipelined.

Sharding: tensor-parallel=4 over kv-head pairs x data-parallel=2 over batch.
Each core: one batch, 8 q-heads, 2 kv-heads, full 2048-token sequence.

Structure (per core):
  Stage A: K proj (+rope), V proj, Q proj for token block 0 (+rope).
  Stage B: per q-chunk qc: attention for 8 heads, software-pipelined:
    iter h: scores(h) [2 matmuls/kc-pair] -> exp pair [ACT, [128,1024] -> bf16]
            attnV(h-1) accumulate, then DVE in-place tree-fold of e(h-1)
            for the softmax denominator; den broadcast via ones[128x128]
            matmul; reciprocal_approx_fast; normalize -> at tile.
    Tensor idle slots are filled from a queue of "filler" units:
    Q projection (+rope) for block qc+1 and O projection for qc-1.
  Softmax denominator: DVE tree-fold (bf16) + one [128x128]-ones matmul
  per (head, qc) producing a broadcast den in PSUM (no gpsimd broadcast,
  no [1,512] DVE reciprocal).
All matmuls bf16 inputs / fp32 PSUM accumulation.
"""
import numpy as np
import ml_dtypes
from collections import deque

import concourse.bacc as bacc
import concourse.bass as bass
import concourse.tile as tile
from concourse import mybir
from concourse.bass_utils import run_bass_kernel_spmd

BF = mybir.dt.bfloat16
F32 = mybir.dt.float32
BF_NP = np.dtype(ml_dtypes.bfloat16)

# full-problem constants
B, S, HIDDEN = 2, 2048, 4096
NUM_HEADS, NUM_KV_HEADS, HEAD_DIM = 32, 8, 128
GROUPS = NUM_HEADS // NUM_KV_HEADS
ROPE_THETA = 10000.0
TP = 4  # shards over kv-head pairs

FULL_CFG = dict(S=2048, HID=4096, NQ=8, NKV=2, SB=512, QC=512)


def build_nc(cfg):
    S_, HID, NQ, NKV, SB, QC = (cfg[k] for k in ("S", "HID", "NQ", "NKV", "SB", "QC"))
    D = 128
    HC = HID // 128          # hidden chunks (contraction tiles)
    NB = S_ // SB            # token blocks (also q-chunks)
    KT = S_ // 128           # k-token tiles
    NP = KT // 2             # kc pairs
    DV = NKV * 128           # local v width
    NO = HID // 512          # O-proj output chunks
    GRP = NQ // NKV
    scale = 1.0 / np.sqrt(128.0)

    nc = bacc.Bacc("TRN2", target_bir_lowering=False, debug=False)
    xt = nc.dram_tensor("xt", (HID, S_), BF, kind="ExternalInput").ap()
    wq = nc.dram_tensor("wq", (HC, NQ, 128, 128), BF, kind="ExternalInput").ap()
    wk = nc.dram_tensor("wk", (HC, NKV, 128, 128), BF, kind="ExternalInput").ap()
    wv = nc.dram_tensor("wv", (HC, 128, DV), BF, kind="ExternalInput").ap()
    wo = nc.dram_tensor("wo", (NQ, NO, 128, 512), BF, kind="ExternalInput").ap()
    cosd = nc.dram_tensor("cos", (128, S_), BF, kind="ExternalInput").ap()
    sind = nc.dram_tensor("sin", (128, S_), BF, kind="ExternalInput").ap()
    rmatd = nc.dram_tensor("rmat", (128, 128), BF, kind="ExternalInput").ap()
    o = nc.dram_tensor("o", (S_, HID), F32, kind="ExternalOutput").ap()

    xt_r = xt.rearrange("(c p) s -> p c s", p=128)

    with tile.TileContext(nc) as tc:
        with tc.tile_pool(name="cons", bufs=1) as cons, \
             tc.tile_pool(name="big", bufs=1) as big:
            cos_sb = cons.tile([128, S_], BF, name="cos_sb")
            sin_sb = cons.tile([128, S_], BF, name="sin_sb")
            r_sb = cons.tile([128, 128], BF, name="r_sb")
            ones_sb = cons.tile([128, 128], BF, name="ones_sb")
            nc.scalar.dma_start(out=cos_sb, in_=cosd)
            nc.scalar.dma_start(out=sin_sb, in_=sind)
            nc.scalar.dma_start(out=r_sb, in_=rmatd)
            nc.vector.memset(ones_sb, 1.0)

            q_sb = big.tile([128, NQ, S_], BF, name="q_sb")
            k_sb = big.tile([128, NKV, S_], BF, name="k_sb")
            v_sb = big.tile([128, KT, DV], BF, name="v_sb")

            # ---------------- stage A: K, V, Q(block0) projections ----------
            with tc.tile_pool(name="xpA", bufs=2) as xpA, \
                 tc.tile_pool(name="wpA", bufs=2) as wpA, \
                 tc.tile_pool(name="wvp", bufs=1) as wvp, \
                 tc.tile_pool(name="qbfA", bufs=2) as qbfA, \
                 tc.tile_pool(name="tpA", bufs=2) as tpA, \
                 tc.tile_pool(name="ppA", bufs=2, space="PSUM") as ppA, \
                 tc.tile_pool(name="rotA", bufs=2, space="PSUM") as rotA:
                wv_sb = wvp.tile([128, HC, DV], BF, name="wv_sb")
                nc.scalar.dma_start(out=wv_sb, in_=wv.rearrange("c p v -> p c v"))

                def rope_combine(ps, rot_pool, qbf_pool, t_pool, dst_ap, ssl):
                    """dst = ps*cos + (R@ps)*sin_folded over token slice ssl."""
                    qbf = qbf_pool.tile([128, SB], BF, name="rope_bf")
                    nc.scalar.activation(out=qbf, in_=ps,
                                         func=mybir.ActivationFunctionType.Copy)
                    rot = rot_pool.tile([128, SB], F32, name="rot_ps")
                    nc.tensor.matmul(rot, r_sb, qbf, start=True, stop=True)
                    t1 = t_pool.tile([128, SB], F32, name="rope_t1")
                    t2 = t_pool.tile([128, SB], F32, name="rope_t2")
                    nc.vector.tensor_mul(t1, ps, cos_sb[:, ssl])
                    nc.vector.tensor_mul(t2, rot, sin_sb[:, ssl])
                    nc.vector.tensor_add(dst_ap, t1, t2)

                for sb_i in range(NB):
                    ssl = slice(sb_i * SB, (sb_i + 1) * SB)
                    xt_t = xpA.tile([128, HC, SB], BF, name="xt_tA")
                    # split the load so the first matmuls start after 1/4 of it
                    for c0 in range(0, HC, 8):
                        nc.sync.dma_start(out=xt_t[:, c0:c0 + 8, :],
                                          in_=xt_r[:, c0:c0 + 8, ssl])

                    heads = [("k", kvh) for kvh in range(NKV)]
                    if sb_i == 0:
                        heads += [("q", h) for h in range(NQ)]
                    for which, h in heads:
                        wten, dst = (wk, k_sb) if which == "k" else (wq, q_sb)
                        wslab = wpA.tile([128, HC, 128], BF, name="w_slabA")
                        nc.sync.dma_start(
                            out=wslab, in_=wten[:, h].rearrange("c p m -> p c m"))
                        ps = ppA.tile([128, SB], F32, name="ps_projA")
                        for c in range(HC):
                            nc.tensor.matmul(ps, wslab[:, c, :], xt_t[:, c, :],
                                             start=(c == 0), stop=(c == HC - 1))
                        rope_combine(ps, rotA, qbfA, tpA, dst[:, h, ssl], ssl)

                    # V projection (natural [tok, d] layout)
                    for tt in range(SB // 128):
                        ps = ppA.tile([128, DV], F32, name="ps_vA")
                        for c in range(HC):
                            nc.tensor.matmul(ps, xt_t[:, c, tt * 128:(tt + 1) * 128],
                                             wv_sb[:, c, :],
                                             start=(c == 0), stop=(c == HC - 1))
                        nc.scalar.activation(out=v_sb[:, sb_i * (SB // 128) + tt, :],
                                             in_=ps,
                                             func=mybir.ActivationFunctionType.Copy)

            # ------------- stage B: attention + Q proj + O proj pipelined ----
            with tc.tile_pool(name="xpB", bufs=1) as xpB, \
                 tc.tile_pool(name="wpB", bufs=2) as wpB, \
                 tc.tile_pool(name="wob", bufs=2) as wob, \
                 tc.tile_pool(name="ep", bufs=2) as ep, \
                 tc.tile_pool(name="atp", bufs=2) as atp, \
                 tc.tile_pool(name="esp", bufs=2) as esp, \
                 tc.tile_pool(name="rcp", bufs=2) as rcp, \
                 tc.tile_pool(name="qbfB", bufs=2) as qbfB, \
                 tc.tile_pool(name="tpB", bufs=2) as tpB, \
                 tc.tile_pool(name="otp", bufs=3) as otp, \
                 tc.tile_pool(name="sp", bufs=1, space="PSUM") as sp, \
                 tc.tile_pool(name="ap_", bufs=2, space="PSUM") as ap_, \
                 tc.tile_pool(name="qr", bufs=2, space="PSUM") as qr, \
                 tc.tile_pool(name="dp", bufs=2, space="PSUM") as dp:

                filler = deque()
                iter_budget = [10 ** 9]

                def pump(n):
                    """Emit ~n matmuls worth of filler work (iter-budgeted)."""
                    n = min(n, iter_budget[0])
                    while n > 0 and filler:
                        try:
                            k = next(filler[0])
                            n -= k
                            iter_budget[0] -= k
                        except StopIteration:
                            filler.popleft()

                def drain_filler():
                    while filler:
                        try:
                            next(filler[0])
                        except StopIteration:
                            filler.popleft()

                xt_cur = [None]

                def xload_unit(b):
                    ssl = slice(b * SB, (b + 1) * SB)
                    xt_t = xpB.tile([128, HC, SB], BF, name="xt_tB")
                    nc.sync.dma_start(out=xt_t, in_=xt_r[:, :, ssl])
                    xt_cur[0] = xt_t
                    yield 0

                def qproj_unit(b, h):
                    ssl = slice(b * SB, (b + 1) * SB)
                    wslab = wpB.tile([128, HC, 128], BF, name="w_slabB")
                    nc.sync.dma_start(
                        out=wslab, in_=wq[:, h].rearrange("c p m -> p c m"))
                    yield 0
                    xt_t = xt_cur[0]
                    ps = qr.tile([128, SB], F32, name="qr_ps")
                    for c0 in range(0, HC, 8):
                        for c in range(c0, c0 + 8):
                            nc.tensor.matmul(ps, wslab[:, c, :], xt_t[:, c, :],
                                             start=(c == 0), stop=(c == HC - 1),
                                             skip_group_check=True)
                        yield 8
                    # rope (t1 emitted before rot-mm so the qps slot frees
                    # without depending on later tensor work)
                    qbf = qbfB.tile([128, SB], BF, name="rope_bfB")
                    nc.scalar.activation(out=qbf, in_=ps,
                                         func=mybir.ActivationFunctionType.Copy)
                    t1 = tpB.tile([128, SB], F32, name="rope_t1B")
                    nc.vector.tensor_mul(t1, ps, cos_sb[:, ssl])
                    yield 4  # let ACT drain qbf before the rot matmul needs it
                    rot = qr.tile([128, SB], F32, name="qr_ps")
                    nc.tensor.matmul(rot, r_sb, qbf, start=True, stop=True)
                    t2 = tpB.tile([128, SB], F32, name="rope_t2B")
                    nc.vector.tensor_mul(t2, rot, sin_sb[:, ssl])
                    nc.vector.tensor_add(q_sb[:, h, ssl], t1, t2)
                    yield 1

                def oproj_unit(qc, n, at_t):
                    qsl0 = qc * QC
                    wos = wob.tile([128, NQ, 512], BF, name="wo_slab")
                    nc.sync.dma_start(
                        out=wos, in_=wo[:, n].rearrange("c p m -> p c m"))
                    yield 0
                    for tt in range(QC // 128):
                        ps_o = dp.tile([128, 512], F32, name="dp_ps")
                        for c in range(NQ):
                            nc.tensor.matmul(
                                ps_o, at_t[:, c, tt * 128:(tt + 1) * 128],
                                wos[:, c, :],
                                start=(c == 0), stop=(c == NQ - 1),
                                skip_group_check=True)
                        yield NQ
                        o_t = otp.tile([128, 512], F32, name="o_t")
                        if tt % 2 == 0:
                            nc.scalar.activation(
                                out=o_t, in_=ps_o,
                                func=mybir.ActivationFunctionType.Copy)
                        else:
                            nc.vector.tensor_copy(o_t, ps_o)
                        nc.scalar.dma_start(
                            out=o[qsl0 + tt * 128: qsl0 + (tt + 1) * 128,
                                  n * 512:(n + 1) * 512],
                            in_=o_t)
                        yield 0

                # pipeline state per in-flight head
                e_tiles = {}      # h -> e tile [128, KT, 512]
                aps_tiles = {}    # h -> attn psum [128, QC]
                esum_tiles = {}   # h -> [128, 512] bf16

                budget_tbl = [28, 55, 55, 27]
                for qc in range(NB):
                    qsl = slice(qc * QC, (qc + 1) * QC)
                    at_t = atp.tile([128, NQ, QC], BF, name="at_t")
                    if qc < NB - 1:
                        filler.append(xload_unit(qc + 1))
                        for h in range(NQ):
                            filler.append(qproj_unit(qc + 1, h))

                    for it in range(NQ + 2):
                        iter_budget[0] = budget_tbl[qc]
                        h_s = it if it < NQ else None          # scores/exp head
                        h_a = it - 1 if 1 <= it <= NQ else None  # attnV/tree head
                        h_d = it - 2 if it >= 2 else None      # den/norm head

                        if h_a is not None:
                            kvh = h_a // GRP
                            aps = ap_.tile([128, QC], F32, name="attn_ps")
                            aps_tiles[h_a] = aps
                        if h_s is not None:
                            kvh_s = h_s // GRP
                            e_t = ep.tile([128, KT, 512], BF, name="e_t")
                            e_tiles[h_s] = e_t
                        for p in range(NP if h_s is not None else 0):
                            s_pair = sp.tile([128, 2, 512], F32, name="s_pair")
                            for j in range(2):
                                kc = 2 * p + j
                                nc.tensor.matmul(
                                    s_pair[:, j, :],
                                    k_sb[:, kvh_s, kc * 128:(kc + 1) * 128],
                                    q_sb[:, h_s, qsl], start=True, stop=True)
                            nc.scalar.activation(
                                out=e_tiles[h_s][:, 2 * p:2 * p + 2, :],
                                in_=s_pair,
                                func=mybir.ActivationFunctionType.Exp, scale=scale)
                            if h_a is not None:
                                for j in range(2):
                                    kc = 2 * p + j
                                    nc.tensor.matmul(
                                        aps_tiles[h_a],
                                        v_sb[:, kc, kvh * 128:(kvh + 1) * 128],
                                        e_tiles[h_a][:, kc, :],
                                        start=(kc == 0), stop=(kc == KT - 1),
                                        skip_group_check=True)
                            pump(8)
                        if h_s is None and h_a is not None:
                            # drain iteration: attnV without concurrent scores
                            for p in range(NP):
                                for j in range(2):
                                    kc = 2 * p + j
                                    nc.tensor.matmul(
                                        aps_tiles[h_a],
                                        v_sb[:, kc, kvh * 128:(kvh + 1) * 128],
                                        e_tiles[h_a][:, kc, :],
                                        start=(kc == 0), stop=(kc == KT - 1),
                                        skip_group_check=True)
                                pump(8)

                        if h_a is not None:
                            # tree-fold e(h_a) in place for the denominator
                            e_t = e_tiles[h_a]
                            nc.vector.tensor_add(e_t[:, 0:4, :], e_t[:, 0:4, :],
                                                 e_t[:, 4:8, :])
                            nc.vector.tensor_add(e_t[:, 8:12, :], e_t[:, 8:12, :],
                                                 e_t[:, 12:16, :])
                            nc.vector.tensor_add(e_t[:, 0:2, :], e_t[:, 0:2, :],
                                                 e_t[:, 2:4, :])
                            nc.vector.tensor_add(e_t[:, 8:10, :], e_t[:, 8:10, :],
                                                 e_t[:, 10:12, :])
                            nc.vector.tensor_add(e_t[:, 0:2, :], e_t[:, 0:2, :],
                                                 e_t[:, 8:10, :])
                            esum = esp.tile([128, 512], BF, name="esum")
                            nc.vector.tensor_add(esum, e_t[:, 0, :], e_t[:, 1, :])
                            esum_tiles[h_a] = esum

                        if h_d is not None:
                            den_bc = qr.tile([128, SB], F32, name="qr_ps")
                            nc.tensor.matmul(den_bc, ones_sb, esum_tiles.pop(h_d),
                                             start=True, stop=True)
                            rec = rcp.tile([128, 512], F32, name="rec")
                            nc.vector.reciprocal_approx_fast(out=rec, in_=den_bc)
                            nc.vector.tensor_mul(at_t[:, h_d, :],
                                                 aps_tiles.pop(h_d), rec)
                            del e_tiles[h_d]
                            pump(4)

                    # O projection for this qc becomes filler for the next
                    for n in range(NO):
                        filler.append(oproj_unit(qc, n, at_t))
                    if qc == NB - 1:
                        iter_budget[0] = 10 ** 9
                        drain_filler()
                iter_budget[0] = 10 ** 9
                drain_filler()
    nc.compile()
    return nc


def _rope_tables(position_ids_b, S_):
    """cos/sin tables in [d=128, s] layout, sin sign-folded for the half-swap."""
    pos = position_ids_b.astype(np.float32)
    inv_freq = (1.0 / (ROPE_THETA ** (np.arange(0, HEAD_DIM, 2, dtype=np.float32)
                                      / HEAD_DIM))).astype(np.float32)
    freqs = pos[:, None] * inv_freq[None, :]          # [s, 64]
    emb = np.concatenate([freqs, freqs], axis=1)      # [s, 128]
    cos = np.cos(emb).T.copy()                        # [128, s]
    sin = np.sin(emb).T.copy()
    sin[:64] *= -1.0                                  # sign-fold for swap rope
    return cos.astype(BF_NP), sin.astype(BF_NP)


def _prep_core_inputs(hidden_states, position_ids, Wq, Wk, Wv, Wo):
    rmat = np.zeros((128, 128), dtype=np.float32)
    for i in range(128):
        rmat[i, (i + 64) % 128] = 1.0
    rmat = rmat.astype(BF_NP)

    HC = HIDDEN // 128
    in_maps = []
    for t in range(TP):
        fq = slice(1024 * t, 1024 * (t + 1))
        fkv = slice(256 * t, 256 * (t + 1))
        wq_t = np.ascontiguousarray(
            Wq[:, fq].reshape(HC, 128, 8, 128).transpose(0, 2, 1, 3)).astype(BF_NP)
        wk_t = np.ascontiguousarray(
            Wk[:, fkv].reshape(HC, 128, 2, 128).transpose(0, 2, 1, 3)).astype(BF_NP)
        wv_t = np.ascontiguousarray(Wv[:, fkv].reshape(HC, 128, 256)).astype(BF_NP)
        wo_t = np.ascontiguousarray(
            Wo[fq, :].reshape(8, 128, 8, 512).transpose(0, 2, 1, 3)).astype(BF_NP)
        for b in range(B):
            xt = np.ascontiguousarray(hidden_states[b].T).astype(BF_NP)
            cos, sin = _rope_tables(position_ids[b], S)
            in_maps.append({"xt": xt, "wq": wq_t, "wk": wk_t, "wv": wv_t,
                            "wo": wo_t, "cos": cos, "sin": sin, "rmat": rmat})
    return in_maps


_NC_CACHE = {}


def kernel(hidden_states, position_ids, Wq, Wk, Wv, Wo):
    if "nc" not in _NC_CACHE:
        _NC_CACHE["nc"] = build_nc(FULL_CFG)
    nc = _NC_CACHE["nc"]
    in_maps = _prep_core_inputs(np.asarray(hidden_states), np.asarray(position_ids),
                                np.asarray(Wq), np.asarray(Wk),
                                np.asarray(Wv), np.asarray(Wo))
    res = run_bass_kernel_spmd(nc, in_maps, core_ids=list(range(8)))
    out = np.zeros((B, S, HIDDEN), dtype=np.float32)
    for t in range(TP):
        for b in range(B):
            out[b] += res.results[t * B + b]["o"]
    return out
